# revision 4
# baseline (speedup 1.0000x reference)
"""KMeans min-distance loss kernel for Trainium2 (8 NeuronCores, SPMD).

Problem: features [262144, 128] f32, centers [256, 128] f32.
  d2[n,k] = ||f_n||^2 + ||c_k||^2 - 2 f_n.c_k ; out = mean_n sqrt(min_k d2)

Sharding: data-parallel over N (32768 rows per core), centers replicated.
Each core returns [128] partial sums of min-distances; host reduces.

Per-core pipeline (per 128-row chunk):
  - DMA 1MB groups of 16 chunks  [128p, 16, 128] f32
  - f2 = sum(f^2) along free dim     (DVE tensor_tensor_reduce mult/add)
  - PE transpose chunk -> featT in PSUM (batches of 4 into one bank)
  - ACT evacuates PSUM -> SBUF featT
  - PE matmul: psum = featT.T @ (-2 centers.T)  -> [128n, 256k]
  - DVE tensor_tensor_reduce: min_k(psum + c2)  -> m[:, chunk]
  - tail: sqrt(max(m + f2, 0)) with ACT accum -> [128] sums -> DMA out
"""

import sys

for p in ("/opt/trn_rl_repo", "/opt/trn_rl_repo/concourse"):
    if p not in sys.path:
        sys.path.insert(0, p)

import numpy as np

N_TOTAL = 262144
K = 256
D = 128
N_CORES = 8
N_PER_CORE = N_TOTAL // N_CORES  # 32768
P = 128
CHUNKS = N_PER_CORE // P         # 256 chunks of 128 rows
G = 16                           # chunks per DMA group (1 MB)
GROUPS = CHUNKS // G             # 16
TG = 4                           # chunks per transpose/evac batch

_compiled = None


def _build(repeat: int = 1):
    import concourse.bass as bass
    import concourse.bacc as bacc
    import concourse.tile as tile
    from concourse import mybir

    f32 = mybir.dt.float32
    Alu = mybir.AluOpType
    Act = mybir.ActivationFunctionType

    nc = bacc.Bacc(
        "TRN2", target_bir_lowering=False, debug=False, num_devices=N_CORES
    )

    feats = nc.dram_tensor("features", [N_PER_CORE, D], f32, kind="ExternalInput").ap()
    ctneg2 = nc.dram_tensor("ctneg2", [D, K], f32, kind="ExternalInput").ap()
    c2r = nc.dram_tensor("c2r", [1, K], f32, kind="ExternalInput").ap()
    ones = nc.dram_tensor("ones", [1, P], f32, kind="ExternalInput").ap()
    ident = nc.dram_tensor("ident", [P, P], f32, kind="ExternalInput").ap()
    out = nc.dram_tensor("out", [P, 1], f32, kind="ExternalOutput").ap()

    with tile.TileContext(nc) as tc:
        with (
            tc.tile_pool(name="consts", bufs=1) as consts,
            tc.tile_pool(name="featg", bufs=3) as featg_pool,
            tc.tile_pool(name="featT", bufs=3) as featT_pool,
            tc.tile_pool(name="dumps", bufs=2) as dumps,
            tc.tile_pool(name="coll", bufs=1) as coll,
            tc.tile_pool(name="ptrans", bufs=2, space="PSUM") as ptrans_pool,
            tc.tile_pool(name="pcross", bufs=4, space="PSUM") as pcross_pool,
        ):
            ct_s = consts.tile([D, K], f32)
            nc.sync.dma_start(ct_s[:], ctneg2)
            c2r_s = consts.tile([1, K], f32)
            nc.sync.dma_start(c2r_s[:], c2r)
            ones_s = consts.tile([1, P], f32)
            nc.sync.dma_start(ones_s[:], ones)
            id_s = consts.tile([P, P], f32)
            nc.sync.dma_start(id_s[:], ident)

            m_coll = coll.tile([P, CHUNKS], f32)
            f2_coll = coll.tile([P, CHUNKS], f32)

            # features viewed as [group, partition, chunk-in-group, d]
            fview = feats.rearrange("(g c p) d -> g p c d", p=P, c=G)

            for g in range(GROUPS * repeat):
                g = g % GROUPS
                fg = featg_pool.tile([P, G, D], f32)
                nc.sync.dma_start(fg[:], fview[g])

                for cb in range(G // TG):
                    pt = ptrans_pool.tile([D, TG * P], f32)
                    for j in range(TG):
                        c = cb * TG + j
                        # PE transpose: psum[:, jP:(j+1)P] = fg[:,c,:].T
                        nc.tensor.transpose(
                            pt[:, bass.ts(j, P)], fg[:, c, :], id_s[:]
                        )
                    fT = featT_pool.tile([D, TG * P], f32)
                    nc.scalar.copy(fT[:], pt[:])

                    for j in range(TG):
                        c = cb * TG + j
                        i = g * G + c
                        # f2 for this chunk (DVE, fused square+sum-accum)
                        d128 = dumps.tile([P, D], f32, tag="d128")
                        nc.vector.scalar_tensor_tensor(
                            out=d128[:],
                            in0=fg[:, c, :],
                            scalar=1.0,
                            in1=fg[:, c, :],
                            op0=Alu.mult,
                            op1=Alu.mult,
                            accum_out=f2_coll[:, i : i + 1],
                        )
                        px = pcross_pool.tile([P, K], f32)
                        # psum = c2[k] (rank-1) + (-2 cross)
                        nc.tensor.matmul(
                            px[:], ones_s[:], c2r_s[:], start=True, stop=False,
                        )
                        nc.tensor.matmul(
                            px[:], fT[:, bass.ts(j, P)], ct_s[:],
                            start=False, stop=True,
                        )
                        nc.vector.tensor_reduce(
                            out=m_coll[:, i : i + 1],
                            in_=px[:],
                            axis=mybir.AxisListType.X,
                            op=Alu.min,
                        )

            # tail: sums[p] = sum_i sqrt(max(m[p,i] + f2[p,i], 0))
            d2t = coll.tile([P, CHUNKS], f32)
            nc.vector.tensor_add(d2t[:], m_coll[:], f2_coll[:])
            nc.vector.tensor_scalar_max(d2t[:], d2t[:], 0.0)
            dist = coll.tile([P, CHUNKS], f32)
            sums = coll.tile([P, 1], f32)
            nc.scalar.activation(
                dist[:], d2t[:], Act.Sqrt, accum_out=sums[:]
            )
            nc.sync.dma_start(out, sums[:])

    nc.compile()
    return nc


def _get_compiled():
    global _compiled
    if _compiled is None:
        _compiled = _build()
    return _compiled


def kernel(features: np.ndarray, centers: np.ndarray) -> np.ndarray:
    features = np.ascontiguousarray(np.asarray(features, dtype=np.float32))
    centers = np.ascontiguousarray(np.asarray(centers, dtype=np.float32))
    assert features.shape == (N_TOTAL, D) and centers.shape == (K, D)

    from concourse.bass_utils import run_bass_kernel_spmd

    nc = _get_compiled()

    ctneg2 = np.ascontiguousarray((-2.0 * centers.T).astype(np.float32))  # [D, K]
    c2 = np.sum(centers.astype(np.float64) ** 2, axis=1).astype(np.float32)  # [K]
    c2r = np.ascontiguousarray(c2[None, :])
    ones = np.ones((1, P), dtype=np.float32)
    ident = np.eye(P, dtype=np.float32)

    in_maps = []
    for c in range(N_CORES):
        in_maps.append(
            {
                "features": features[c * N_PER_CORE : (c + 1) * N_PER_CORE],
                "ctneg2": ctneg2,
                "c2r": c2r,
                "ones": ones,
                "ident": ident,
            }
        )

    res = run_bass_kernel_spmd(nc, in_maps, list(range(N_CORES)))
    total = 0.0
    for r in res.results:
        total += np.sum(r["out"].astype(np.float64))
    return np.float32(total / N_TOTAL)


if __name__ == "__main__":
    rng = np.random.default_rng(0)
    f = rng.standard_normal((N_TOTAL, D), dtype=np.float32)
    c = rng.standard_normal((K, D), dtype=np.float32)
    print(kernel(f, c))


# revision 5
# speedup vs baseline: 1.0434x; 1.0434x over previous
"""KMeans min-distance loss kernel for Trainium2 (8 NeuronCores, SPMD).

Problem: features [262144, 128] f32, centers [256, 128] f32.
  d2[n,k] = ||f_n||^2 + ||c_k||^2 - 2 f_n.c_k ; out = mean_n sqrt(min_k d2)

Sharding: data-parallel over N (32768 rows per core), centers replicated.
Each core returns [128] partial sums of min-distances; host reduces.

Per-core pipeline (bf16 compute, f32 accumulate):
  - SWDGE cast-DMA 1MB groups: f32 dram -> bf16 sbuf [128p, 16, 128]
  - PE transpose (bf16) chunks -> featT, batches of 4 per PSUM bank
  - ACT evacuates PSUM -> SBUF featT
  - PE: rank-1 fp16 matmul preloads centered ||c||^2 into PSUM, then
    bf16 cross matmuls accumulate -2 f.c  -> [128n, 4, 256k]
  - DVE segmented tensor_reduce min over k -> m[:, 4]
  - f2 = sum(f^2): alternates DVE scalar_tensor_tensor / ACT Square+accum
  - tail: sqrt(m + f2 + mean_c2) with ACT accum -> [128] sums -> DMA out
"""

import sys

for p in ("/opt/trn_rl_repo", "/opt/trn_rl_repo/concourse"):
    if p not in sys.path:
        sys.path.insert(0, p)

import numpy as np

N_TOTAL = 262144
K = 256
D = 128
N_CORES = 8
N_PER_CORE = N_TOTAL // N_CORES  # 32768
P = 128
CHUNKS = N_PER_CORE // P         # 256 chunks of 128 rows
G = 16                           # chunks per DMA group (1 MB f32 read)
GROUPS = CHUNKS // G             # 16
TG = 4                           # chunks per transpose/psum/reduce batch

_compiled = None


def _build(repeat: int = 1):
    import concourse.bass as bass
    import concourse.bacc as bacc
    import concourse.tile as tile
    from concourse import mybir

    f32 = mybir.dt.float32
    bf16 = mybir.dt.bfloat16
    fp16 = mybir.dt.float16
    Alu = mybir.AluOpType
    Act = mybir.ActivationFunctionType

    nc = bacc.Bacc(
        "TRN2", target_bir_lowering=False, debug=False, num_devices=N_CORES
    )

    feats = nc.dram_tensor("features", [N_PER_CORE, D], f32, kind="ExternalInput").ap()
    ctneg2 = nc.dram_tensor("ctneg2", [D, K], bf16, kind="ExternalInput").ap()
    c2q = nc.dram_tensor("c2q", [1, TG * K], fp16, kind="ExternalInput").ap()
    ones = nc.dram_tensor("ones", [1, P], fp16, kind="ExternalInput").ap()
    ident = nc.dram_tensor("ident", [P, P], bf16, kind="ExternalInput").ap()
    c2mean = nc.dram_tensor("c2mean", [P, 1], f32, kind="ExternalInput").ap()
    out = nc.dram_tensor("out", [P, 1], f32, kind="ExternalOutput").ap()

    with tile.TileContext(nc) as tc:
        with (
            tc.tile_pool(name="consts", bufs=1) as consts,
            tc.tile_pool(name="featg", bufs=3) as featg_pool,
            tc.tile_pool(name="featT", bufs=4) as featT_pool,
            tc.tile_pool(name="dumps", bufs=2) as dumps,
            tc.tile_pool(name="coll", bufs=1) as coll,
            tc.tile_pool(name="ptrans", bufs=2, space="PSUM") as ptrans_pool,
            tc.tile_pool(name="pcross", bufs=3, space="PSUM") as pcross_pool,
        ):
            ct_s = consts.tile([D, K], bf16)
            nc.sync.dma_start(ct_s[:], ctneg2)
            c2q_s = consts.tile([1, TG * K], fp16)
            nc.sync.dma_start(c2q_s[:], c2q)
            ones_s = consts.tile([1, P], fp16)
            nc.sync.dma_start(ones_s[:], ones)
            id_s = consts.tile([P, P], bf16)
            nc.sync.dma_start(id_s[:], ident)
            c2m_s = consts.tile([P, 1], f32)
            nc.sync.dma_start(c2m_s[:], c2mean)

            m_coll = coll.tile([P, CHUNKS], f32)
            f2_coll = coll.tile([P, CHUNKS], f32)

            # features viewed as [group, partition, chunk-in-group, d]
            fview = feats.rearrange("(g c p) d -> g p c d", p=P, c=G)

            for g in range(GROUPS * repeat):
                g = g % GROUPS
                fg = featg_pool.tile([P, G, D], bf16)
                nc.gpsimd.dma_start(fg[:], fview[g])  # SWDGE cast f32->bf16

                for cb in range(G // TG):
                    pt = ptrans_pool.tile([D, TG * P], bf16)
                    for j in range(TG):
                        c = cb * TG + j
                        nc.tensor.transpose(
                            pt[:, bass.ts(j, P)], fg[:, c, :], id_s[:]
                        )
                    fT = featT_pool.tile([D, TG * P], bf16)
                    nc.scalar.copy(fT[:], pt[:])

                    px4 = pcross_pool.tile([P, TG, K], f32)
                    px4f = px4[:].rearrange("p c k -> p (c k)")
                    for h in range(2):
                        nc.tensor.matmul(
                            px4f[:, bass.ts(h, TG * K // 2)],
                            ones_s[:],
                            c2q_s[:, bass.ts(h, TG * K // 2)],
                            start=True, stop=False, skip_group_check=True,
                        )
                    for j in range(TG):
                        c = cb * TG + j
                        i = g * G + c
                        # f2: alternate DVE / ACT to balance engines
                        if j % 2 == 0:
                            d128 = dumps.tile([P, D], bf16, tag="d128")
                            nc.vector.scalar_tensor_tensor(
                                out=d128[:],
                                in0=fg[:, c, :],
                                scalar=1.0,
                                in1=fg[:, c, :],
                                op0=Alu.mult,
                                op1=Alu.mult,
                                accum_out=f2_coll[:, i : i + 1],
                            )
                        else:
                            dA = dumps.tile([P, D], bf16, tag="dA")
                            nc.scalar.activation(
                                dA[:], fg[:, c, :], Act.Square,
                                accum_out=f2_coll[:, i : i + 1],
                            )
                        nc.tensor.matmul(
                            px4[:, j, :], fT[:, bass.ts(j, P)], ct_s[:],
                            start=False, stop=(j == TG - 1),
                            skip_group_check=True,
                        )
                    ib = g * G + cb * TG
                    nc.vector.tensor_reduce(
                        out=m_coll[:, ib : ib + TG],
                        in_=px4[:],
                        axis=mybir.AxisListType.X,
                        op=Alu.min,
                    )

            # tail: sums[p] = sum_i sqrt(m[p,i] + f2[p,i] + c2mean)
            d2t = coll.tile([P, CHUNKS], f32)
            nc.vector.tensor_add(d2t[:], m_coll[:], f2_coll[:])
            dist = coll.tile([P, CHUNKS], f32)
            sums = coll.tile([P, 1], f32)
            nc.scalar.activation(
                dist[:], d2t[:], Act.Sqrt, bias=c2m_s[:], accum_out=sums[:]
            )
            nc.sync.dma_start(out, sums[:])

    nc.compile()
    return nc


def _get_compiled():
    global _compiled
    if _compiled is None:
        _compiled = _build()
    return _compiled


def _make_aux(centers: np.ndarray):
    import ml_dtypes

    cen_bf = centers.astype(ml_dtypes.bfloat16)
    ctneg2 = np.ascontiguousarray(
        (-2.0 * cen_bf.astype(np.float32).T)
    ).astype(ml_dtypes.bfloat16)                                   # [D, K]
    c2 = (cen_bf.astype(np.float64) ** 2).sum(axis=1)              # [K]
    c2m = float(c2.mean())
    c2c = (c2 - c2m).astype(np.float16)
    c2q = np.ascontiguousarray(np.tile(c2c[None, :], (1, TG)))     # [1, TG*K]
    ones = np.ones((1, P), dtype=np.float16)
    ident = np.eye(P, dtype=ml_dtypes.bfloat16)
    c2mean = np.full((P, 1), c2m, dtype=np.float32)
    return ctneg2, c2q, ones, ident, c2mean


def _make_in_maps(features: np.ndarray, centers: np.ndarray):
    ctneg2, c2q, ones, ident, c2mean = _make_aux(centers)
    return [
        {
            "features": features[c * N_PER_CORE : (c + 1) * N_PER_CORE],
            "ctneg2": ctneg2,
            "c2q": c2q,
            "ones": ones,
            "ident": ident,
            "c2mean": c2mean,
        }
        for c in range(N_CORES)
    ]


def kernel(features: np.ndarray, centers: np.ndarray) -> np.ndarray:
    features = np.ascontiguousarray(np.asarray(features, dtype=np.float32))
    centers = np.ascontiguousarray(np.asarray(centers, dtype=np.float32))
    assert features.shape == (N_TOTAL, D) and centers.shape == (K, D)

    from concourse.bass_utils import run_bass_kernel_spmd

    nc = _get_compiled()
    in_maps = _make_in_maps(features, centers)
    res = run_bass_kernel_spmd(nc, in_maps, list(range(N_CORES)))
    total = 0.0
    for r in res.results:
        total += np.sum(r["out"].astype(np.float64))
    return np.float32(total / N_TOTAL)


if __name__ == "__main__":
    rng = np.random.default_rng(0)
    f = rng.standard_normal((N_TOTAL, D), dtype=np.float32)
    c = rng.standard_normal((K, D), dtype=np.float32)
    print(kernel(f, c))


# revision 6
# speedup vs baseline: 2.8315x; 2.7138x over previous
"""KMeans min-distance loss kernel for Trainium2 (8 NeuronCores, SPMD).

Problem: features [262144, 128] f32, centers [256, 128] f32.
  d2[n,k] = ||f_n||^2 + ||c_k||^2 - 2 f_n.c_k ; out = mean_n sqrt(min_k d2)

Sharding: data-parallel over N (32768 rows per core), centers replicated.
Each core returns [128] partial sums of min-distances; host reduces.

Per-core pipeline (bf16 compute, f32 accumulate):
  - SWDGE cast-DMA 1MB groups: f32 dram -> bf16 sbuf [128p, 16, 128]
  - PE transpose (bf16) chunks -> featT, batches of 4 per PSUM bank
  - ACT evacuates PSUM -> SBUF featT
  - PE: rank-1 fp16 matmul preloads centered ||c||^2 into PSUM, then
    bf16 cross matmuls accumulate -2 f.c  -> [128n, 4, 256k]
  - DVE segmented tensor_reduce min over k -> m[:, 4]
  - f2 = sum(f^2): alternates DVE scalar_tensor_tensor / ACT Square+accum
  - tail: sqrt(m + f2 + mean_c2) with ACT accum -> [128] sums -> DMA out
"""

import sys

for p in ("/opt/trn_rl_repo", "/opt/trn_rl_repo/concourse"):
    if p not in sys.path:
        sys.path.insert(0, p)

import numpy as np

N_TOTAL = 262144
K = 256
D = 128
N_CORES = 8
N_PER_CORE = N_TOTAL // N_CORES  # 32768
P = 128
CHUNKS = N_PER_CORE // P         # 256 chunks of 128 rows
G = 16                           # chunks per DMA group (1 MB f32 read)
GROUPS = CHUNKS // G             # 16
TG = 4                           # chunks per transpose/psum/reduce batch

_compiled = None


def _build(repeat: int = 1):
    import concourse.bass as bass
    import concourse.bacc as bacc
    import concourse.tile as tile
    from concourse import mybir

    f32 = mybir.dt.float32
    bf16 = mybir.dt.bfloat16
    fp16 = mybir.dt.float16
    Alu = mybir.AluOpType
    Act = mybir.ActivationFunctionType

    nc = bacc.Bacc(
        "TRN2", target_bir_lowering=False, debug=False, num_devices=N_CORES
    )

    feats = nc.dram_tensor("features", [N_PER_CORE, D], f32, kind="ExternalInput").ap()
    ctneg2 = nc.dram_tensor("ctneg2", [D, K], bf16, kind="ExternalInput").ap()
    c2q = nc.dram_tensor("c2q", [1, TG * K], fp16, kind="ExternalInput").ap()
    ones = nc.dram_tensor("ones", [1, P], fp16, kind="ExternalInput").ap()
    ident = nc.dram_tensor("ident", [P, P], bf16, kind="ExternalInput").ap()
    c2mean = nc.dram_tensor("c2mean", [P, 1], f32, kind="ExternalInput").ap()
    out = nc.dram_tensor("out", [P, 1], f32, kind="ExternalOutput").ap()

    with tile.TileContext(nc) as tc:
        with (
            tc.tile_pool(name="consts", bufs=1) as consts,
            tc.tile_pool(name="featg", bufs=3) as featg_pool,
            tc.tile_pool(name="featT", bufs=4) as featT_pool,
            tc.tile_pool(name="dumps", bufs=2) as dumps,
            tc.tile_pool(name="coll", bufs=1) as coll,
            tc.tile_pool(name="ptrans", bufs=2, space="PSUM") as ptrans_pool,
            tc.tile_pool(name="pcross", bufs=3, space="PSUM") as pcross_pool,
        ):
            ct_s = consts.tile([D, K], bf16)
            nc.sync.dma_start(ct_s[:], ctneg2)
            c2q_s = consts.tile([1, TG * K], fp16)
            nc.sync.dma_start(c2q_s[:], c2q)
            ones_s = consts.tile([1, P], fp16)
            nc.sync.dma_start(ones_s[:], ones)
            id_s = consts.tile([P, P], bf16)
            nc.sync.dma_start(id_s[:], ident)
            c2m_s = consts.tile([P, 1], f32)
            nc.sync.dma_start(c2m_s[:], c2mean)

            m_coll = coll.tile([P, CHUNKS], f32)
            f2_coll = coll.tile([P, CHUNKS], f32)

            # features viewed as [group, partition, chunk-in-group, d].
            # Partition p takes G consecutive rows (one 8KB contiguous
            # descriptor per partition); chunk->row mapping is permuted,
            # which the order-invariant sum tolerates.
            fview = feats.rearrange("(g p c) d -> g p c d", p=P, c=G)

            for g in range(GROUPS * repeat):
                g = g % GROUPS
                fg = featg_pool.tile([P, G, D], bf16)
                nc.gpsimd.dma_start(fg[:], fview[g])  # SWDGE cast f32->bf16

                for cb in range(G // TG):
                    pt = ptrans_pool.tile([D, TG * P], bf16)
                    for j in range(TG):
                        c = cb * TG + j
                        nc.tensor.transpose(
                            pt[:, bass.ts(j, P)], fg[:, c, :], id_s[:]
                        )
                    fT = featT_pool.tile([D, TG * P], bf16)
                    nc.scalar.copy(fT[:], pt[:])

                    px4 = pcross_pool.tile([P, TG, K], f32)
                    px4f = px4[:].rearrange("p c k -> p (c k)")
                    for h in range(2):
                        nc.tensor.matmul(
                            px4f[:, bass.ts(h, TG * K // 2)],
                            ones_s[:],
                            c2q_s[:, bass.ts(h, TG * K // 2)],
                            start=True, stop=False, skip_group_check=True,
                        )
                    for j in range(TG):
                        c = cb * TG + j
                        i = g * G + c
                        # f2: alternate DVE / ACT to balance engines
                        if j % 2 == 0:
                            d128 = dumps.tile([P, D], bf16, tag="d128")
                            nc.vector.scalar_tensor_tensor(
                                out=d128[:],
                                in0=fg[:, c, :],
                                scalar=1.0,
                                in1=fg[:, c, :],
                                op0=Alu.mult,
                                op1=Alu.mult,
                                accum_out=f2_coll[:, i : i + 1],
                            )
                        else:
                            dA = dumps.tile([P, D], bf16, tag="dA")
                            nc.scalar.activation(
                                dA[:], fg[:, c, :], Act.Square,
                                accum_out=f2_coll[:, i : i + 1],
                            )
                        nc.tensor.matmul(
                            px4[:, j, :], fT[:, bass.ts(j, P)], ct_s[:],
                            start=False, stop=(j == TG - 1),
                            skip_group_check=True,
                        )
                    ib = g * G + cb * TG
                    nc.vector.tensor_reduce(
                        out=m_coll[:, ib : ib + TG],
                        in_=px4[:],
                        axis=mybir.AxisListType.X,
                        op=Alu.min,
                    )

            # tail: sums[p] = sum_i sqrt(m[p,i] + f2[p,i] + c2mean)
            d2t = coll.tile([P, CHUNKS], f32)
            nc.vector.tensor_add(d2t[:], m_coll[:], f2_coll[:])
            dist = coll.tile([P, CHUNKS], f32)
            sums = coll.tile([P, 1], f32)
            nc.scalar.activation(
                dist[:], d2t[:], Act.Sqrt, bias=c2m_s[:], accum_out=sums[:]
            )
            nc.sync.dma_start(out, sums[:])

    nc.compile()
    return nc


def _get_compiled():
    global _compiled
    if _compiled is None:
        _compiled = _build()
    return _compiled


def _make_aux(centers: np.ndarray):
    import ml_dtypes

    cen_bf = centers.astype(ml_dtypes.bfloat16)
    ctneg2 = np.ascontiguousarray(
        (-2.0 * cen_bf.astype(np.float32).T)
    ).astype(ml_dtypes.bfloat16)                                   # [D, K]
    c2 = (cen_bf.astype(np.float64) ** 2).sum(axis=1)              # [K]
    c2m = float(c2.mean())
    c2c = (c2 - c2m).astype(np.float16)
    c2q = np.ascontiguousarray(np.tile(c2c[None, :], (1, TG)))     # [1, TG*K]
    ones = np.ones((1, P), dtype=np.float16)
    ident = np.eye(P, dtype=ml_dtypes.bfloat16)
    c2mean = np.full((P, 1), c2m, dtype=np.float32)
    return ctneg2, c2q, ones, ident, c2mean


def _make_in_maps(features: np.ndarray, centers: np.ndarray):
    ctneg2, c2q, ones, ident, c2mean = _make_aux(centers)
    return [
        {
            "features": features[c * N_PER_CORE : (c + 1) * N_PER_CORE],
            "ctneg2": ctneg2,
            "c2q": c2q,
            "ones": ones,
            "ident": ident,
            "c2mean": c2mean,
        }
        for c in range(N_CORES)
    ]


def kernel(features: np.ndarray, centers: np.ndarray) -> np.ndarray:
    features = np.ascontiguousarray(np.asarray(features, dtype=np.float32))
    centers = np.ascontiguousarray(np.asarray(centers, dtype=np.float32))
    assert features.shape == (N_TOTAL, D) and centers.shape == (K, D)

    from concourse.bass_utils import run_bass_kernel_spmd

    nc = _get_compiled()
    in_maps = _make_in_maps(features, centers)
    res = run_bass_kernel_spmd(nc, in_maps, list(range(N_CORES)))
    total = 0.0
    for r in res.results:
        total += np.sum(r["out"].astype(np.float64))
    return np.float32(total / N_TOTAL)


if __name__ == "__main__":
    rng = np.random.default_rng(0)
    f = rng.standard_normal((N_TOTAL, D), dtype=np.float32)
    c = rng.standard_normal((K, D), dtype=np.float32)
    print(kernel(f, c))
